# revision 2
# baseline (speedup 1.0000x reference)
"""Channel-attention MultiHeadAttention kernel for Trainium2 (8 NeuronCores).

Math: for this module, attention is over channels (d x d per head) with the
spatial dim N = H*W as the contraction axis. The whole module collapses:
  G = x @ x.T (256x256 Gram), s = rowsum(x)
  S = scale * [Wq|bq] @ [[G, s],[s^T, N]] @ [Wk|bk]^T   (only 8 diag 32x32 blocks)
  attn = softmax(S_blocks)
  Wfinal = WoutP @ blockdiag(attn) @ Wv ;  bfinal = bout + WoutP @ blockdiag(attn) @ bv
  out = Wfinal @ x + bfinal
Sharding: data-parallel over batch B=8, one batch element per core. No collectives.

I/O in fp16 both ways (host casts): x arrives fp16 (8 MB/core) and is DMA'd
straight into persistent SBUF tiles; out leaves fp16 (8 MB/core) and the host
upcasts. This halves HBM traffic vs fp32 and removes all on-device casts of x.
"""

import numpy as np
from contextlib import ExitStack

B, C, H, W = 8, 256, 128, 128
N = H * W          # 16384
NH, D = 8, 32      # heads, head dim
SCALE = D ** -0.5
CH = 512           # phase C column chunk (one PSUM bank fp32)
SUB = 128          # transpose subchunk
NCORES = 8

TRACE = False      # test.py may set kernel.TRACE = True
LAST_RESULTS = {}  # exec_time_ns etc. for test.py

_CACHE = {}


def _build_real(repeat=1):
    import concourse.bacc as bacc
    import concourse.mybir as mybir
    import concourse.tile as tile

    dt = mybir.dt
    f32, f16 = dt.float32, dt.float16
    Exp = mybir.ActivationFunctionType.Exp
    Ident = mybir.ActivationFunctionType.Identity
    X = mybir.AxisListType.X

    nc = bacc.Bacc(trn_type="TRN2")

    x_d = nc.dram_tensor("xb", [C, N], f16, kind="ExternalInput")
    qaT_d = nc.dram_tensor("qaT", [257, 256], f16, kind="ExternalInput")
    kaT_d = nc.dram_tensor("kaT", [257, 256], f16, kind="ExternalInput")
    wva_d = nc.dram_tensor("wva", [256, 257], f16, kind="ExternalInput")
    wpT_d = nc.dram_tensor("wpT", [256, 256], f16, kind="ExternalInput")
    bout_d = nc.dram_tensor("boutc", [256, 1], f32, kind="ExternalInput")
    id_d = nc.dram_tensor("ident", [128, 128], f16, kind="ExternalInput")
    gd_d = nc.dram_tensor("gdiag", [256, 257], f32, kind="ExternalInput")
    gd1_d = nc.dram_tensor("gdiag1", [128, 129], f32, kind="ExternalInput")
    corr_d = nc.dram_tensor("corr", [256, 32], f32, kind="ExternalInput")
    out_d = nc.dram_tensor("out", [C, N], f16, kind="ExternalOutput")

    with ExitStack() as top:
        tc = top.enter_context(tile.TileContext(nc))
        persist = top.enter_context(tc.tile_pool(name="persist", bufs=1))

        x16 = [persist.tile([128, N], f16, tag=f"x16_{i}", name=f"x16_{i}") for i in range(2)]

        qaT_t = [persist.tile([128, 256], f16, tag="qaT0", name="qaT0"),
                 persist.tile([128, 256], f16, tag="qaT1", name="qaT1"),
                 persist.tile([1, 256], f16, tag="qaT2", name="qaT2")]
        kaT_t = [persist.tile([128, 256], f16, tag="kaT0", name="kaT0"),
                 persist.tile([128, 256], f16, tag="kaT1", name="kaT1"),
                 persist.tile([1, 256], f16, tag="kaT2", name="kaT2")]
        wva_t = [persist.tile([128, 257], f16, tag="wva0", name="wva0"),
                 persist.tile([128, 257], f16, tag="wva1", name="wva1")]
        wpT_t = [persist.tile([128, 256], f16, tag="wpT0", name="wpT0"),
                 persist.tile([128, 256], f16, tag="wpT1", name="wpT1")]
        bout_t = [persist.tile([128, 1], f32, tag="bout0", name="bout0"),
                  persist.tile([128, 1], f32, tag="bout1", name="bout1")]
        id16 = persist.tile([128, 128], f16, tag="id16", name="id16")
        gd_t = [persist.tile([128, 257], f32, tag="gd0", name="gd0"),
                persist.tile([128, 129], f32, tag="gd1", name="gd1")]
        corr_t = [persist.tile([128, 32], f32, tag="corr0", name="corr0"),
                  persist.tile([128, 32], f32, tag="corr1", name="corr1")]

        nc.scalar.dma_start(out=qaT_t[0], in_=qaT_d.ap()[0:128, :])
        nc.scalar.dma_start(out=qaT_t[1], in_=qaT_d.ap()[128:256, :])
        nc.scalar.dma_start(out=qaT_t[2], in_=qaT_d.ap()[256:257, :])
        nc.scalar.dma_start(out=kaT_t[0], in_=kaT_d.ap()[0:128, :])
        nc.scalar.dma_start(out=kaT_t[1], in_=kaT_d.ap()[128:256, :])
        nc.scalar.dma_start(out=kaT_t[2], in_=kaT_d.ap()[256:257, :])
        nc.scalar.dma_start(out=wva_t[0], in_=wva_d.ap()[0:128, :])
        nc.scalar.dma_start(out=wva_t[1], in_=wva_d.ap()[128:256, :])
        nc.scalar.dma_start(out=wpT_t[0], in_=wpT_d.ap()[0:128, :])
        nc.scalar.dma_start(out=wpT_t[1], in_=wpT_d.ap()[128:256, :])
        nc.scalar.dma_start(out=bout_t[0], in_=bout_d.ap()[0:128, :])
        nc.scalar.dma_start(out=bout_t[1], in_=bout_d.ap()[128:256, :])
        nc.scalar.dma_start(out=id16, in_=id_d.ap())
        nc.scalar.dma_start(out=gd_t[0], in_=gd_d.ap()[0:128, :])
        nc.scalar.dma_start(out=gd_t[1], in_=gd1_d.ap())
        nc.scalar.dma_start(out=corr_t[0], in_=corr_d.ap()[0:128, :])
        nc.scalar.dma_start(out=corr_t[1], in_=corr_d.ap()[128:256, :])

        # outputs of the tiny stage used by phase C
        wf16 = [persist.tile([128, 256], f16, tag=f"wf16_{k}", name=f"wf16_{k}") for k in range(2)]
        bf_col = [persist.tile([128, 1], f32, tag=f"bf{m}", name=f"bf{m}") for m in range(2)]

        # ---------------- Phase A: Gram accumulation ----------------
        rep = ExitStack()
        if repeat > 1:
            rep.enter_context(tc.For_i(0, repeat, 1))
        with ExitStack() as sA:
            psA = sA.enter_context(tc.tile_pool(name="psA", bufs=1, space="PSUM"))
            psT = sA.enter_context(tc.tile_pool(name="psT", bufs=4, space="PSUM"))

            G_ps = [psA.tile([128, 257], f32, tag="g0", name="g0"),
                    psA.tile([128, 129], f32, tag="g1", name="g1")]

            # explicit ring of transpose-target tiles (ones column pre-set)
            NXT = 8
            xts = [persist.tile([128, 257], f16, tag=f"xt{j}", name=f"xt{j}") for j in range(NXT)]
            for j in range(NXT):
                nc.vector.memset(xts[j][:, 256:257], 1.0)

            t = 0
            LCH = 2048
            for co in range(N // LCH):
                sl = slice(co * LCH, (co + 1) * LCH)
                # fp16 x straight into the persistent SBUF copy (no cast)
                nc.sync.dma_start(out=x16[0][:, sl], in_=x_d.ap()[0:128, sl])
                nc.sync.dma_start(out=x16[1][:, sl], in_=x_d.ap()[128:256, sl])
                for ci in range(LCH // SUB):
                    n0 = co * LCH + ci * SUB
                    xt = xts[t % NXT]
                    # PE transpose (fp16, 1 cy/row) of both channel halves
                    tp_ps = psT.tile([128, 256], f16, tag="tps", name="tps")
                    nc.tensor.transpose(tp_ps[:, 0:128],
                                        x16[0][:, n0:n0 + SUB], id16[:])
                    nc.tensor.transpose(tp_ps[:, 128:256],
                                        x16[1][:, n0:n0 + SUB], id16[:])
                    # PSUM -> SBUF copy: 5/8 on DVE, 3/8 on ACT
                    if t % 8 in (1, 4, 6):
                        nc.scalar.copy(out=xt[:, 0:256], in_=tp_ps[:])
                    else:
                        nc.vector.tensor_copy(out=xt[:, 0:256], in_=tp_ps[:])
                    t += 1
                    first, last = (n0 == 0), (n0 == N - SUB)
                    nc.tensor.matmul(G_ps[0][:], lhsT=xt[:, 0:128], rhs=xt[:],
                                     start=first, stop=last)
                    nc.tensor.matmul(G_ps[1][:], lhsT=xt[:, 128:256],
                                     rhs=xt[:, 128:257],
                                     start=first, stop=last)

            # ------- Tiny stage (fp16 matmuls @1cy/row; G diag pre-subtracted,
            # exact correction  scale*N*Qa@Ka^T  added back from host) -------
            tp = sA.enter_context(tc.tile_pool(name="tinysb", bufs=1))
            pst = sA.enter_context(tc.tile_pool(name="tinyps", bufs=2, space="PSUM"))

            # Ga tiles: [G - N*I | s] rows 0:256 plus the [s^T | 0] row, fp16
            Ga = [tp.tile([128, 257], f16, tag=f"Ga{k}", name=f"Ga{k}") for k in range(2)]
            nc.vector.tensor_sub(Ga[0][:], G_ps[0][:], gd_t[0][:])
            # G10 = G01^T (symmetry); G11/s1 from the 129-col accumulator
            g10_ps = pst.tile([128, 128], f16, tag="tinyps", name="g10ps")
            nc.tensor.transpose(g10_ps[:], Ga[0][:, 128:256], id16[:])
            nc.scalar.copy(out=Ga[1][:, 0:128], in_=g10_ps[:])
            nc.vector.tensor_sub(Ga[1][:, 128:257], G_ps[1][:, 0:129], gd_t[1][:])
            Ga2 = tp.tile([1, 257], f16, tag="Ga2", name="Ga2")
            for k in range(2):
                srow_ps = pst.tile([1, 128], f16, tag="tinyps", name="tinyps")
                nc.tensor.transpose(srow_ps[:], Ga[k][:, 256:257], id16[:])
                nc.vector.tensor_copy(out=Ga2[0:1, 128 * k:128 * (k + 1)],
                                      in_=srow_ps[:])
            nc.vector.memset(Ga2[0:1, 256:257], 0.0)
            GaK = [Ga[0], Ga[1], Ga2]

            # T2 = Ga' @ KaT  (257 x 256), M-tiles over rows of T2
            t2s = [tp.tile([128, 256], f16, tag="t2s0", name="t2s0"),
                   tp.tile([128, 256], f16, tag="t2s1", name="t2s1"),
                   tp.tile([1, 256], f16, tag="t2s2", name="t2s2")]
            for m in range(3):
                msl = slice(256, 257) if m == 2 else slice(128 * m, 128 * (m + 1))
                t2_ps = pst.tile([1 if m == 2 else 128, 256], f32, tag="tinyps", name="tinyps")
                for k in range(3):
                    nc.tensor.matmul(t2_ps[:], lhsT=GaK[k][:, msl], rhs=kaT_t[k][:],
                                     start=(k == 0), stop=(k == 2))
                if m == 1:
                    nc.scalar.copy(out=t2s[m][:], in_=t2_ps[:])
                else:
                    nc.vector.tensor_copy(out=t2s[m][:], in_=t2_ps[:])

            # S_full' = QaT.T @ T2 (256 x 256) in PSUM
            SF = []
            for m in range(2):
                sf_ps = pst.tile([128, 256], f32, tag="tinyps", name=f"sfps{m}")
                msl = slice(128 * m, 128 * (m + 1))
                for k in range(3):
                    nc.tensor.matmul(sf_ps[:], lhsT=qaT_t[k][:, msl], rhs=t2s[k][:],
                                     start=(k == 0), stop=(k == 2))
                SF.append(sf_ps)

            # extract diag 32x32 blocks + add correction -> S_stack (2 x (128, 32))
            Sst = [tp.tile([128, 32], f32, tag=f"sst{q}", name=f"sst{q}") for q in range(2)]
            for h in range(NH):
                q, po = h // 4, (h % 4) * 32
                nc.vector.tensor_add(
                    Sst[q][po:po + 32, :],
                    SF[q][po:po + 32, h * 32:(h + 1) * 32],
                    corr_t[q][po:po + 32, :])

            # softmax over free dim (32)
            att = [tp.tile([128, 32], f32, tag=f"att{q}", name=f"att{q}") for q in range(2)]
            for q in range(2):
                nm = tp.tile([128, 1], f32, tag=f"nm{q}", name=f"nm{q}")
                nc.vector.reduce_max(out=nm[:], in_=Sst[q][:], axis=X, negate=True)
                ex = tp.tile([128, 32], f32, tag=f"ex{q}", name=f"ex{q}")
                nc.scalar.activation(out=ex[:], in_=Sst[q][:], func=Exp,
                                     bias=nm[:], scale=1.0)
                sm = tp.tile([128, 1], f32, tag=f"sm{q}", name=f"sm{q}")
                nc.vector.reduce_sum(out=sm[:], in_=ex[:], axis=X)
                rc = tp.tile([128, 1], f32, tag=f"rc{q}", name=f"rc{q}")
                nc.vector.reciprocal(out=rc[:], in_=sm[:])
                nc.vector.tensor_scalar_mul(att[q][:], ex[:], rc[:])

            # block-diagonal attn (fp16), transpose the two diagonal quadrants
            abd = [tp.tile([128, 128], f16, tag=f"abd{q}", name=f"abd{q}") for q in range(2)]
            for q in range(2):
                nc.gpsimd.memset(abd[q][:], 0.0)
            for h in range(NH):
                q, po = h // 4, (h % 4) * 32
                nc.vector.tensor_copy(out=abd[q][po:po + 32, po:po + 32],
                                      in_=att[q][po:po + 32, :])
            abdT = [tp.tile([128, 128], f16, tag=f"abdT{q}", name=f"abdT{q}") for q in range(2)]
            for q in range(2):
                tq_ps = pst.tile([128, 128], f16, tag="tinyps", name="tinyps")
                nc.tensor.transpose(tq_ps[:], abd[q][:], id16[:])
                if q == 1:
                    nc.scalar.copy(out=abdT[q][:], in_=tq_ps[:])
                else:
                    nc.vector.tensor_copy(out=abdT[q][:], in_=tq_ps[:])

            # Weff_aug = blockdiag(attn) @ [Wv | bv]  (256 x 257) fp16
            weff = [tp.tile([128, 257], f16, tag=f"weff{k}", name=f"weff{k}") for k in range(2)]
            for k in range(2):
                we_ps = pst.tile([128, 257], f32, tag="tinyps", name="tinyps")
                nc.tensor.matmul(we_ps[:], lhsT=abdT[k][:], rhs=wva_t[k][:],
                                 start=True, stop=True)
                if k == 1:
                    nc.scalar.copy(out=weff[k][:], in_=we_ps[:])
                else:
                    nc.vector.tensor_copy(out=weff[k][:], in_=we_ps[:])

            # Wfinal^T = Weff[:, :256] as lhsT against WoutP^T; cast to fp16
            for m in range(2):
                msl = slice(128 * m, 128 * (m + 1))
                wf_ps = pst.tile([128, 256], f32, tag="tinyps", name=f"wfps{m}")
                for k in range(2):
                    nc.tensor.matmul(wf_ps[:], lhsT=weff[k][:, msl], rhs=wpT_t[k][:],
                                     start=(k == 0), stop=(k == 1))
                if m == 1:
                    nc.scalar.copy(out=wf16[m][:], in_=wf_ps[:])
                else:
                    nc.vector.tensor_copy(out=wf16[m][:], in_=wf_ps[:])

            # bfinal = bout + WoutP @ beff   (beff = Weff[:, 256])
            for m in range(2):
                msl = slice(128 * m, 128 * (m + 1))
                bf_ps = pst.tile([128, 1], f32, tag="tinyps", name="tinyps")
                for k in range(2):
                    nc.tensor.matmul(bf_ps[:], lhsT=wpT_t[k][:, msl],
                                     rhs=weff[k][:, 256:257],
                                     start=(k == 0), stop=(k == 1))
                nc.vector.tensor_add(bf_col[m][:], bf_ps[:], bout_t[m][:])

        # ---------------- Phase C: out = Wfinal @ x + bfinal ----------------
        with ExitStack() as sC:
            ost = sC.enter_context(tc.tile_pool(name="ost", bufs=3))
            psC = sC.enter_context(tc.tile_pool(name="psC", bufs=6, space="PSUM"))
            SCH = 2048
            for co in range(N // SCH):
                slo = slice(co * SCH, (co + 1) * SCH)
                for m in range(2):
                    o_sb = ost.tile([128, SCH], f16, tag=f"osb{m}", name=f"osb{m}")
                    for h in range(SCH // CH):
                        sl = slice(co * SCH + h * CH, co * SCH + (h + 1) * CH)
                        o_ps = psC.tile([128, CH], f32, tag="ops", name="ops")
                        for k in range(2):
                            nc.tensor.matmul(o_ps[:], lhsT=wf16[k][:, 128 * m:128 * (m + 1)],
                                             rhs=x16[k][:, sl],
                                             start=(k == 0), stop=(k == 1))
                        dst = o_sb[:, h * CH:(h + 1) * CH]
                        # cast fp32 PSUM -> fp16 SBUF with bias: alternate DVE/ACT
                        if (co * 2 + h + m) % 2 == 0:
                            nc.vector.tensor_scalar_add(dst, o_ps[:], bf_col[m][:])
                        else:
                            nc.scalar.activation(
                                out=dst, in_=o_ps[:], func=Ident,
                                bias=bf_col[m][:], scale=1.0)
                    nc.sync.dma_start(out=out_d.ap()[128 * m:128 * (m + 1), slo],
                                      in_=o_sb[:])

        rep.close()

    nc.finalize()
    return nc


def _host_prep(Wqkv, bqkv, Wout, bout):
    Wq, Wk, Wv = Wqkv[:C], Wqkv[C:2 * C], Wqkv[2 * C:]
    bq, bk, bv = bqkv[:C], bqkv[C:2 * C], bqkv[2 * C:]
    qa = np.concatenate([Wq, bq[:, None]], axis=1) * SCALE      # (256, 257)
    ka = np.concatenate([Wk, bk[:, None]], axis=1)              # (256, 257)
    qaT = np.ascontiguousarray(qa.T)                            # (257, 256)
    kaT = np.ascontiguousarray(ka.T)
    wva = np.concatenate([Wv, bv[:, None]], axis=1)             # (256, 257)
    r = np.arange(C)
    WoutP = Wout[:, (r % D) * NH + (r // D)]                    # (256, 256)
    wpT = np.ascontiguousarray(WoutP.T)
    gdiag = np.zeros((256, 257), dtype=np.float32)
    gdiag[np.arange(256), np.arange(256)] = float(N)
    gdiag1 = np.zeros((128, 129), dtype=np.float32)
    gdiag1[np.arange(128), np.arange(128)] = float(N)
    corr_full = float(N) * (qa @ ka.T)                          # (256, 256) fp32
    corr = np.zeros((256, 32), dtype=np.float32)
    for h in range(NH):
        corr[h * D:(h + 1) * D, :] = corr_full[h * D:(h + 1) * D,
                                               h * D:(h + 1) * D]
    return {
        "qaT": qaT.astype(np.float16), "kaT": kaT.astype(np.float16),
        "wva": np.ascontiguousarray(wva, dtype=np.float16),
        "wpT": wpT.astype(np.float16),
        "boutc": np.ascontiguousarray(bout[:, None], dtype=np.float32),
        "ident": np.eye(128, dtype=np.float16),
        "gdiag": gdiag, "gdiag1": gdiag1, "corr": corr,
    }


def kernel(x, Wqkv, bqkv, Wout, bout, num_heads):
    from concourse.bass_utils import run_bass_kernel_spmd

    assert int(num_heads) == NH
    x16 = np.asarray(x, dtype=np.float16)
    shared = _host_prep(
        np.asarray(Wqkv, dtype=np.float32), np.asarray(bqkv, dtype=np.float32),
        np.asarray(Wout, dtype=np.float32), np.asarray(bout, dtype=np.float32))

    if "nc" not in _CACHE:
        _CACHE["nc"] = _build_real()
    nc = _CACHE["nc"]

    in_maps = [{"xb": np.ascontiguousarray(x16[c].reshape(C, N)), **shared}
               for c in range(NCORES)]

    res = run_bass_kernel_spmd(nc, in_maps, core_ids=list(range(NCORES)),
                               trace=TRACE)
    LAST_RESULTS["exec_time_ns"] = res.exec_time_ns
    out = np.stack([res.results[c]["out"].astype(np.float32)
                    for c in range(NCORES)])
    return out.reshape(B, C, H, W)
